# revision 1
# baseline (speedup 1.0000x reference)
import numpy as np
import jax
import jax.numpy as jnp
from functools import partial

DIM = 256
HEADS = 8
DIM_HEAD = 64
INNER = HEADS * DIM_HEAD  # 512
DPG = DIM // HEADS        # 32
EPS = 1e-5
N_CORES = 8

_cache = {}


def _get_fn():
    if "fn" not in _cache:
        devs = jax.devices()[:N_CORES]
        scale = DIM_HEAD ** (-0.5)

        @partial(
            jax.pmap,
            axis_name="i",
            devices=devs,
            in_axes=(0, None, None, None, None, None, None, None),
        )
        def run(xs, a, bb, Wq, Wk, Wv, Wout, bout):
            # xs: [P, k, DIM] shard of flattened (b*p) points
            xn = xs * a + bb  # BatchNorm folded to per-channel affine
            P, k, d = xn.shape
            xg = xn.reshape(P, k, HEADS, DPG)
            q = jnp.einsum("pkhc,hoc->phko", xg, Wq)
            kk = jnp.einsum("pkhc,hoc->phko", xg, Wk)
            v = jnp.einsum("pkhc,hoc->phko", xg, Wv)
            dots = jnp.einsum("phid,phjd->phij", q, kk) * scale
            attn = jax.nn.softmax(dots, axis=-1)
            out = jnp.einsum("phij,phjd->phid", attn, v)
            out = out.transpose(0, 2, 1, 3).reshape(P, k, INNER)
            return out @ Wout + bout

        _cache["fn"] = run
    return _cache["fn"]


def kernel(x, bn_gamma, bn_beta, Wq, Wk, Wv, Wout, bout):
    b, p, k, d = x.shape
    xs = np.asarray(x, np.float32).reshape(N_CORES, (b * p) // N_CORES, k, d)

    # BatchNorm2d training-mode batch stats over (b, p, k), folded into a
    # per-channel affine so the device pass reads x exactly once.
    xf = xs.reshape(-1, d)
    nvals = xf.shape[0]
    s = np.einsum("ij->j", xf, dtype=np.float64)
    ss = np.einsum("ij,ij->j", xf, xf, dtype=np.float64)
    mean = s / nvals
    var = ss / nvals - mean * mean
    a = (np.asarray(bn_gamma, np.float64) / np.sqrt(var + EPS)).astype(np.float32)
    bb = (np.asarray(bn_beta, np.float64) - mean * a).astype(np.float32)

    run = _get_fn()
    ys = run(
        xs,
        jnp.asarray(a),
        jnp.asarray(bb),
        jnp.asarray(Wq, jnp.float32),
        jnp.asarray(Wk, jnp.float32),
        jnp.asarray(Wv, jnp.float32),
        jnp.asarray(Wout, jnp.float32),
        jnp.asarray(bout, jnp.float32),
    )
    y = np.asarray(ys).reshape(b, p, k, DIM)
    return np.ascontiguousarray(y, dtype=np.float32)

